# revision 2
# baseline (speedup 1.0000x reference)
"""Neighbor aggregation (gnn message passing) Bass kernel for Trainium2.

out[b, i] = sum_{e: src[e]==i} w[e] * H[b, dst[e]]   (per batch b)

8 NeuronCores: core = 2*b + s handles batch b, src-half s (output rows
[s*25000, (s+1)*25000)).

Strategy: SWDGE dma_gather (per-edge descriptor generation on the Pool
engine, ~2.9 ns/token) is the hard floor; everything else is arranged to
stay far below it so the wall clock is pure Pool time:
 - H is stored bf16 padded to 128 cols (256B rows = the SWDGE minimum
   elem); gathered messages feed the Tensor engine DIRECTLY.  The old
   kernel's DVE MULTIPLY (w * msgs, 217us) and f32->bf16 conversion are
   gone - the edge weight is folded into the one-hot instead.
 - Bins are 32 sources wide (cap 256 tokens per phase, TPB=2).  The
   weighted one-hot ("whot") is built on DVE 32 slots wide in batches of
   16 tiles: is_equal(slot-id, iota32) then mult by w.  At 32 wide this
   is ~4x cheaper than the old 128-wide IS_EQ (893us -> ~250us), taking
   DVE off the critical path entirely (old kernel: DVE 95% busy,
   co-critical with Pool).
 - 4 bins share one PSUM tile at partition offsets 32*(bin%4) via
   explicit matmul tile_position; a DVE copy drains each 4-bin group to
   a staging tile, DMA'd to HBM every 8 bins.  The two dst-half phases
   write separate dumps; the host adds them during the un-permute.
 - Same chunk ramp / 4-SWDGE-queue rotation as before (keeps descriptor
   generation at its measured best rate).
"""

import os
import sys

sys.path.insert(0, "/opt/trn_rl_repo")

import numpy as np
import ml_dtypes

import concourse.bacc as bacc
import concourse.mybir as mybir
import concourse.tile as tile
from concourse.bass_utils import run_bass_kernel_spmd

B, N, E, HS = 4, 50000, 800000, 64
NHALF = N // 2                  # 25000 output rows per core
CH = 8192                       # tokens per gather chunk
TPB = 2                         # tiles (of 128 tokens) per bin per phase
CAP = TPB * 128                 # 256 tokens per bin per phase
W = 32                          # sources (slots) per bin

LAST_RESULT = {}


def _chunk_sizes(nbins, nch_per_phase):
    """Per-phase chunk sizes: graduated ramp first resp. last so the 4 SWDGE
    queue contexts fill/drain staggered instead of all big generations
    starting at once.  One mid-stream chunk is trimmed so the region covers
    exactly nbins*CAP tokens (no dummy-tile pad)."""
    trim = nch_per_phase * CH - nbins * CAP         # multiple of 128, < CH
    ramp = [CH // 4] * 4 + [CH // 2] * 2            # = 2*CH
    sizes_a = ramp + [CH] * (nch_per_phase - 3) + [CH - trim]
    sizes_b = [CH - trim] + [CH] * (nch_per_phase - 3) + ramp[::-1]
    sizes_a = [s for s in sizes_a if s > 0]
    sizes_b = [s for s in sizes_b if s > 0]
    return sizes_a, sizes_b


def build(nc, nbins, nch_per_phase):
    f32 = mybir.dt.float32
    bf16 = mybir.dt.bfloat16
    i16 = mybir.dt.int16
    sizes_a, sizes_b = _chunk_sizes(nbins, nch_per_phase)
    sizes = sizes_a + sizes_b
    nch = len(sizes)
    nquad = nbins // 4

    h_d = nc.dram_tensor("h", [N, 128], bf16, kind="ExternalInput")
    gidx_d = nc.dram_tensor("gidx", [nch, 128, CH // 16], i16,
                            kind="ExternalInput")
    wl_d = nc.dram_tensor("wl", [nch, 128, CH // 128], bf16,
                          kind="ExternalInput")
    scol_d = nc.dram_tensor("scol", [nch, 128, CH // 128], bf16,
                            kind="ExternalInput")
    iotab_d = nc.dram_tensor("iotab", [128, W], bf16, kind="ExternalInput")
    acc_d = nc.dram_tensor("acc", [128, 2, nquad, HS], f32,
                           kind="ExternalOutput")

    with tile.TileContext(nc) as tc:
        with tc.tile_pool(name="res", bufs=1) as res, \
             tc.tile_pool(name="psum", bufs=8, space="PSUM") as pp, \
             tc.tile_pool(name="wk", bufs=4) as wk, \
             tc.tile_pool(name="oh", bufs=4) as ohp, \
             tc.tile_pool(name="st", bufs=2) as stp:
            iotab = res.tile([128, W], bf16, tag="iotab")
            nc.sync.dma_start(iotab[:], iotab_d[:])

            ps = None
            stage = None
            off = 0                     # token offset within the phase
            for c, size in enumerate(sizes):
                phase = 0 if c < len(sizes_a) else 1
                if c == len(sizes_a):
                    off = 0
                h_ap = h_d[:][phase * NHALF:(phase + 1) * NHALF, :]
                ntile = size // 128
                gi = wk.tile([128, size // 16], i16, tag="gi")
                nc.scalar.dma_start(gi[:], gidx_d[c][:, :size // 16])
                wl = wk.tile([128, ntile], bf16, tag="wl")
                nc.scalar.dma_start(wl[:], wl_d[c][:, :ntile])
                sc = wk.tile([128, ntile], bf16, tag="sc")
                nc.scalar.dma_start(sc[:], scol_d[c][:, :ntile])

                msgs = wk.tile([128, ntile, 128], bf16, tag="msgs")
                nc.gpsimd.dma_gather(
                    out_ap=msgs[:],
                    in_ap=h_ap,
                    idxs_ap=gi[:],
                    num_idxs=size,
                    num_idxs_reg=size,
                    elem_size=128,
                    single_packet=False,
                    queue_num=c % 4,
                )

                tau0 = off // 128
                for j0 in range(0, ntile, 16):
                    nb = min(16, ntile - j0)
                    oh = ohp.tile([128, 16, W], bf16, tag="oh")
                    nc.vector.tensor_tensor(
                        out=oh[:][:, :nb, :],
                        in0=sc[:][:, j0:j0 + nb].unsqueeze(2)
                            .broadcast_to([128, nb, W]),
                        in1=iotab[:].unsqueeze(1).broadcast_to([128, nb, W]),
                        op=mybir.AluOpType.is_equal,
                    )
                    nc.vector.tensor_tensor(
                        out=oh[:][:, :nb, :],
                        in0=oh[:][:, :nb, :],
                        in1=wl[:][:, j0:j0 + nb].unsqueeze(2)
                            .broadcast_to([128, nb, W]),
                        op=mybir.AluOpType.mult,
                    )
                    for j in range(j0, j0 + nb):
                        tau = tau0 + j              # tile idx in phase
                        bin_, pos = tau // TPB, tau % TPB
                        k = bin_ % 4
                        if pos == 0 and k == 0:
                            ps = pp.tile([128, HS], f32, tag="ps")
                        nc.tensor.matmul(
                            ps[:][32 * k:32 * k + 32, :],
                            oh[:][:, j - j0, :],
                            msgs[:][:, j, 0:HS],
                            start=(pos == 0),
                            stop=(pos == TPB - 1),
                            tile_position=(0, 32 * k),
                        )
                        if pos == TPB - 1 and k == 3:
                            quad = bin_ // 4
                            if quad % 2 == 0:
                                stage = stp.tile([128, 2, HS], f32,
                                                 tag="stage")
                            nc.vector.tensor_scalar_add(
                                stage[:][:, quad % 2, :], ps[:], 0.0)
                            if quad % 2 == 1:
                                nc.sync.dma_start(
                                    acc_d[:][:, phase, quad - 1:quad + 1, :],
                                    stage[:],
                                )
                off += size
    return nc


_COMPILED = {}


def _get_compiled(nbins, nch_per_phase):
    key = (nbins, nch_per_phase)
    if key not in _COMPILED:
        nc = bacc.Bacc(
            "TRN2", target_bir_lowering=False, debug=False, num_swdge_queues=4
        )
        build(nc, nbins, nch_per_phase)
        nc.compile()
        _COMPILED[key] = nc
    return _COMPILED[key]


def _pack_bins(dA, dB, nbins):
    """Assign each source to a bin s.t. per-bin source count <= W and
    per-bin token sums <= CAP in BOTH phases."""
    nsrc = dA.shape[0]
    order = np.argsort(-(dA + dB), kind="stable")
    loadA = np.zeros(nbins, np.int64)
    loadB = np.zeros(nbins, np.int64)
    cnt = np.zeros(nbins, np.int64)
    bin_of = np.empty(nsrc, np.int64)
    slot_of = np.empty(nsrc, np.int64)
    for s in order:
        headA = CAP - loadA - dA[s]
        headB = CAP - loadB - dB[s]
        score = np.minimum(headA, headB)
        score[cnt >= W] = -1
        b = int(np.argmax(score))
        if score[b] < 0:
            return None
        bin_of[s] = b
        slot_of[s] = cnt[b]
        loadA[b] += dA[s]
        loadB[b] += dB[s]
        cnt[b] += 1
    return bin_of, slot_of


def _wrap16(idx, n):
    a = idx.reshape(n // 16, 16).T.astype(np.int16)   # [16, n//16]
    return np.ascontiguousarray(np.tile(a, (8, 1)))   # [128, n//16]


def _core_edges(src, dst, w, s):
    sel = (src >= NHALF) == bool(s)
    srcs = (src[sel] - s * NHALF).astype(np.int64)
    dsts = dst[sel].astype(np.int64)
    ws = w[sel].astype(np.float32)
    phase = (dsts >= NHALF).astype(np.int64)
    dloc = dsts - phase * NHALF
    return srcs, dloc, ws, phase


def _prep_core(srcs, dloc, ws, phase, bin_of, slot_of, nbins, nch_per_phase):
    """Build gidx/wl/scol chunk arrays for one core (batch half s)."""
    ntok = nch_per_phase * CH
    g_all = np.zeros((2, ntok), np.int64)
    w_all = np.zeros((2, ntok), np.float32)
    s_all = np.full((2, ntok), -1.0, np.float32)

    for ph in range(2):
        m = phase == ph
        sp, dp, wp_ = srcs[m], dloc[m], ws[m]
        b = bin_of[sp]
        order = np.argsort(b, kind="stable")
        sp, dp, wp_, b = sp[order], dp[order], wp_[order], b[order]
        cnts = np.bincount(b, minlength=nbins)
        starts = np.concatenate([[0], np.cumsum(cnts[:-1])])
        offs = np.arange(sp.shape[0]) - np.repeat(starts, cnts)
        pos = b * CAP + offs
        assert (offs < CAP).all()
        g_all[ph, pos] = dp
        w_all[ph, pos] = wp_
        s_all[ph, pos] = slot_of[sp]

    sizes_a, sizes_b = _chunk_sizes(nbins, nch_per_phase)
    nch = len(sizes_a) + len(sizes_b)
    gidx = np.zeros((nch, 128, CH // 16), np.int16)
    wl = np.zeros((nch, 128, CH // 128), ml_dtypes.bfloat16)
    scol = np.full((nch, 128, CH // 128), -1.0, ml_dtypes.bfloat16)
    c = 0
    for ph, sizes in ((0, sizes_a), (1, sizes_b)):
        off = 0
        for size in sizes:
            tok = slice(off, off + size)
            gidx[c, :, :size // 16] = _wrap16(g_all[ph, tok], size)
            # token t of chunk -> [t % 128, t // 128]
            wl[c, :, :size // 128] = (
                w_all[ph, tok].reshape(size // 128, 128).T
                .astype(ml_dtypes.bfloat16))
            scol[c, :, :size // 128] = (
                s_all[ph, tok].reshape(size // 128, 128).T
                .astype(ml_dtypes.bfloat16))
            off += size
            c += 1
    return {"gidx": gidx, "wl": wl, "scol": scol}


def kernel(**inputs):
    H = np.ascontiguousarray(np.asarray(inputs["H"], np.float32))
    w = np.asarray(inputs["edge_w"], np.float32)
    src = np.asarray(inputs["edge_src"], np.int64)
    dst = np.asarray(inputs["edge_dst"], np.int64)

    edges = []
    worst = 1
    for core in range(8):
        b, s = core // 2, core % 2
        srcs, dloc, ws, phase = _core_edges(src[b], dst[b], w[b], s)
        edges.append((srcs, dloc, ws, phase))
        worst = max(worst, int((phase == 0).sum()), int((phase == 1).sum()))

    # pack all cores; grow nbins until feasible everywhere
    nbins = max(-(-NHALF // W), -(-int(worst * 1.01) // CAP))
    nbins = -(-nbins // 8) * 8
    while True:
        metas = []
        for core in range(8):
            srcs, dloc, ws, phase = edges[core]
            dA = np.bincount(srcs[phase == 0], minlength=NHALF)
            dB = np.bincount(srcs[phase == 1], minlength=NHALF)
            res = _pack_bins(dA, dB, nbins)
            if res is None:
                break
            metas.append(res)
        if len(metas) == 8:
            break
        nbins += 8
    nch_per_phase = -(-(nbins * CAP) // CH)

    iotab = np.tile(np.arange(W), (128, 1)).astype(ml_dtypes.bfloat16)

    in_maps = []
    for core in range(8):
        b = core // 2
        srcs, dloc, ws, phase = edges[core]
        bin_of, slot_of = metas[core]
        m = _prep_core(srcs, dloc, ws, phase, bin_of, slot_of, nbins,
                       nch_per_phase)
        hb = np.zeros((N, 128), ml_dtypes.bfloat16)
        hb[:, 0:HS] = H[b].astype(ml_dtypes.bfloat16)
        m["h"] = hb
        m["iotab"] = iotab
        in_maps.append(m)

    nc = _get_compiled(nbins, nch_per_phase)
    trace = bool(int(os.environ.get("GNN_TRACE", "0")))
    res = run_bass_kernel_spmd(nc, in_maps, list(range(8)), trace=trace)
    LAST_RESULT["exec_time_ns"] = res.exec_time_ns
    LAST_RESULT["res"] = res

    out = np.empty((B, N, HS), np.float32)
    rows = np.arange(NHALF)
    for core in range(8):
        b, s = core // 2, core % 2
        bin_of, slot_of = metas[core]
        dump = np.asarray(res.results[core]["acc"], np.float32)
        # acc [128, 2, nquad, 64]: row src -> partition 32*(bin%4)+slot,
        # quad bin//4; phases add.
        part = 32 * (bin_of[rows] % 4) + slot_of[rows]
        quad = bin_of[rows] // 4
        out[b, s * NHALF:(s + 1) * NHALF] = (
            dump[part, 0, quad] + dump[part, 1, quad])
    return out


# revision 4
# speedup vs baseline: 1.0202x; 1.0202x over previous
"""Neighbor aggregation (gnn message passing) Bass kernel for Trainium2.

out[b, i] = sum_{e: src[e]==i} w[e] * H[b, dst[e]]   (per batch b)

8 NeuronCores: core = 2*b + s handles batch b, src-half s (output rows
[s*25000, (s+1)*25000)).

Strategy: SWDGE dma_gather (per-edge descriptor generation on the Pool
engine, ~2.9 ns/token) is the hard floor; everything else is arranged to
stay far below it so the wall clock is pure Pool time:
 - H is stored bf16 padded to 128 cols (256B rows = the SWDGE minimum
   elem); gathered messages feed the Tensor engine DIRECTLY.  The old
   kernel's DVE MULTIPLY (w * msgs, 217us) and f32->bf16 conversion are
   gone - the edge weight is folded into the one-hot instead.
 - Bins are 32 sources wide (cap 256 tokens per phase, TPB=2).  The
   weighted one-hot ("whot") is built on DVE 32 slots wide in batches of
   16 tiles: is_equal(slot-id, iota32) then mult by w.  At 32 wide this
   is ~4x cheaper than the old 128-wide IS_EQ (893us -> ~250us), taking
   DVE off the critical path entirely (old kernel: DVE 95% busy,
   co-critical with Pool).
 - 4 bins share one PSUM tile at partition offsets 32*(bin%4) via
   explicit matmul tile_position; a DVE copy drains each 4-bin group to
   a staging tile, DMA'd to HBM every 8 bins.  The two dst-half phases
   write separate dumps; the host adds them during the un-permute.
 - Same chunk ramp / 4-SWDGE-queue rotation as before (keeps descriptor
   generation at its measured best rate).
"""

import os
import sys

sys.path.insert(0, "/opt/trn_rl_repo")

import numpy as np
import ml_dtypes

import concourse.bacc as bacc
import concourse.mybir as mybir
import concourse.tile as tile
from concourse.bass_utils import run_bass_kernel_spmd

B, N, E, HS = 4, 50000, 800000, 64
NHALF = N // 2                  # 25000 output rows per core
CH = 12288                      # tokens per gather chunk (Q7 scratch caps
                                # dma_gather at <16368 idxs: 4B/idx of 64KB)
TPB = 2                         # tiles (of 128 tokens) per bin per phase
CAP = TPB * 128                 # 256 tokens per bin per phase
W = 32                          # sources (slots) per bin

LAST_RESULT = {}


def _chunk_sizes(nbins, nch_per_phase):
    """Per-phase chunk sizes: graduated ramp first resp. last so the 4 SWDGE
    queue contexts fill/drain staggered instead of all big generations
    starting at once.  One mid-stream chunk is trimmed so the region covers
    exactly nbins*CAP tokens (no dummy-tile pad)."""
    trim = nch_per_phase * CH - nbins * CAP         # multiple of 128, < CH
    ramp = [CH // 4] * 4 + [CH // 2] * 2            # = 2*CH
    sizes_a = ramp + [CH] * (nch_per_phase - 3) + [CH - trim]
    sizes_b = [CH - trim] + [CH] * (nch_per_phase - 3) + ramp[::-1]
    sizes_a = [s for s in sizes_a if s > 0]
    sizes_b = [s for s in sizes_b if s > 0]
    return sizes_a, sizes_b


def build(nc, nbins, nch_per_phase):
    f32 = mybir.dt.float32
    bf16 = mybir.dt.bfloat16
    i16 = mybir.dt.int16
    sizes_a, sizes_b = _chunk_sizes(nbins, nch_per_phase)
    sizes = sizes_a + sizes_b
    nch = len(sizes)
    nquad = nbins // 4

    h_d = nc.dram_tensor("h", [N, 128], bf16, kind="ExternalInput")
    gidx_d = nc.dram_tensor("gidx", [nch, 128, CH // 16], i16,
                            kind="ExternalInput")
    wl_d = nc.dram_tensor("wl", [nch, 128, CH // 128], bf16,
                          kind="ExternalInput")
    scol_d = nc.dram_tensor("scol", [nch, 128, CH // 128], bf16,
                            kind="ExternalInput")
    iotab_d = nc.dram_tensor("iotab", [128, W], bf16, kind="ExternalInput")
    acc_d = nc.dram_tensor("acc", [128, 2, nquad, HS], f32,
                           kind="ExternalOutput")

    with tile.TileContext(nc) as tc:
        with tc.tile_pool(name="res", bufs=1) as res, \
             tc.tile_pool(name="psum", bufs=8, space="PSUM") as pp, \
             tc.tile_pool(name="wk", bufs=4) as wk, \
             tc.tile_pool(name="oh", bufs=4) as ohp, \
             tc.tile_pool(name="st", bufs=2) as stp:
            iotab = res.tile([128, W], bf16, tag="iotab")
            nc.sync.dma_start(iotab[:], iotab_d[:])

            ps = None
            stage = None
            off = 0                     # token offset within the phase
            for c, size in enumerate(sizes):
                phase = 0 if c < len(sizes_a) else 1
                if c == len(sizes_a):
                    off = 0
                h_ap = h_d[:][phase * NHALF:(phase + 1) * NHALF, :]
                ntile = size // 128
                gi = wk.tile([128, size // 16], i16, tag="gi")
                nc.scalar.dma_start(gi[:], gidx_d[c][:, :size // 16])
                wl = wk.tile([128, ntile], bf16, tag="wl")
                nc.scalar.dma_start(wl[:], wl_d[c][:, :ntile])
                sc = wk.tile([128, ntile], bf16, tag="sc")
                nc.scalar.dma_start(sc[:], scol_d[c][:, :ntile])

                msgs = wk.tile([128, ntile, 128], bf16, tag="msgs")
                nc.gpsimd.dma_gather(
                    out_ap=msgs[:],
                    in_ap=h_ap,
                    idxs_ap=gi[:],
                    num_idxs=size,
                    num_idxs_reg=size,
                    elem_size=128,
                    single_packet=False,
                    queue_num=c % 4,
                )

                tau0 = off // 128
                for j0 in range(0, ntile, 32):
                    nb = min(32, ntile - j0)
                    oh = ohp.tile([128, 32, W], bf16, tag="oh")
                    nc.vector.tensor_tensor(
                        out=oh[:][:, :nb, :],
                        in0=sc[:][:, j0:j0 + nb].unsqueeze(2)
                            .broadcast_to([128, nb, W]),
                        in1=iotab[:].unsqueeze(1).broadcast_to([128, nb, W]),
                        op=mybir.AluOpType.is_equal,
                    )
                    nc.vector.tensor_tensor(
                        out=oh[:][:, :nb, :],
                        in0=oh[:][:, :nb, :],
                        in1=wl[:][:, j0:j0 + nb].unsqueeze(2)
                            .broadcast_to([128, nb, W]),
                        op=mybir.AluOpType.mult,
                    )
                    for j in range(j0, j0 + nb):
                        tau = tau0 + j              # tile idx in phase
                        bin_, pos = tau // TPB, tau % TPB
                        k = bin_ % 4
                        if pos == 0 and k == 0:
                            ps = pp.tile([128, HS], f32, tag="ps")
                        nc.tensor.matmul(
                            ps[:][32 * k:32 * k + 32, :],
                            oh[:][:, j - j0, :],
                            msgs[:][:, j, 0:HS],
                            start=(pos == 0),
                            stop=(pos == TPB - 1),
                            tile_position=(0, 32 * k),
                        )
                        if pos == TPB - 1 and k == 3:
                            quad = bin_ // 4
                            if quad % 2 == 0:
                                stage = stp.tile([128, 2, HS], f32,
                                                 tag="stage")
                            nc.vector.tensor_scalar_add(
                                stage[:][:, quad % 2, :], ps[:], 0.0)
                            if quad % 2 == 1:
                                nc.sync.dma_start(
                                    acc_d[:][:, phase, quad - 1:quad + 1, :],
                                    stage[:],
                                )
                off += size
    return nc


_COMPILED = {}


def _get_compiled(nbins, nch_per_phase):
    key = (nbins, nch_per_phase)
    if key not in _COMPILED:
        nc = bacc.Bacc(
            "TRN2", target_bir_lowering=False, debug=False, num_swdge_queues=4
        )
        build(nc, nbins, nch_per_phase)
        nc.compile()
        _COMPILED[key] = nc
    return _COMPILED[key]


def _pack_bins(dA, dB, nbins):
    """Assign each source to a bin s.t. per-bin source count <= W and
    per-bin token sums <= CAP in BOTH phases."""
    nsrc = dA.shape[0]
    order = np.argsort(-(dA + dB), kind="stable")
    loadA = np.zeros(nbins, np.int64)
    loadB = np.zeros(nbins, np.int64)
    cnt = np.zeros(nbins, np.int64)
    bin_of = np.empty(nsrc, np.int64)
    slot_of = np.empty(nsrc, np.int64)
    for s in order:
        headA = CAP - loadA - dA[s]
        headB = CAP - loadB - dB[s]
        score = np.minimum(headA, headB)
        score[cnt >= W] = -1
        b = int(np.argmax(score))
        if score[b] < 0:
            return None
        bin_of[s] = b
        slot_of[s] = cnt[b]
        loadA[b] += dA[s]
        loadB[b] += dB[s]
        cnt[b] += 1
    return bin_of, slot_of


def _wrap16(idx, n):
    a = idx.reshape(n // 16, 16).T.astype(np.int16)   # [16, n//16]
    return np.ascontiguousarray(np.tile(a, (8, 1)))   # [128, n//16]


def _core_edges(src, dst, w, s):
    sel = (src >= NHALF) == bool(s)
    srcs = (src[sel] - s * NHALF).astype(np.int64)
    dsts = dst[sel].astype(np.int64)
    ws = w[sel].astype(np.float32)
    phase = (dsts >= NHALF).astype(np.int64)
    dloc = dsts - phase * NHALF
    return srcs, dloc, ws, phase


def _prep_core(srcs, dloc, ws, phase, bin_of, slot_of, nbins, nch_per_phase):
    """Build gidx/wl/scol chunk arrays for one core (batch half s)."""
    ntok = nch_per_phase * CH
    g_all = np.zeros((2, ntok), np.int64)
    w_all = np.zeros((2, ntok), np.float32)
    s_all = np.full((2, ntok), -1.0, np.float32)

    for ph in range(2):
        m = phase == ph
        sp, dp, wp_ = srcs[m], dloc[m], ws[m]
        b = bin_of[sp]
        order = np.argsort(b, kind="stable")
        sp, dp, wp_, b = sp[order], dp[order], wp_[order], b[order]
        cnts = np.bincount(b, minlength=nbins)
        starts = np.concatenate([[0], np.cumsum(cnts[:-1])])
        offs = np.arange(sp.shape[0]) - np.repeat(starts, cnts)
        pos = b * CAP + offs
        assert (offs < CAP).all()
        g_all[ph, pos] = dp
        w_all[ph, pos] = wp_
        s_all[ph, pos] = slot_of[sp]

    sizes_a, sizes_b = _chunk_sizes(nbins, nch_per_phase)
    nch = len(sizes_a) + len(sizes_b)
    gidx = np.zeros((nch, 128, CH // 16), np.int16)
    wl = np.zeros((nch, 128, CH // 128), ml_dtypes.bfloat16)
    scol = np.full((nch, 128, CH // 128), -1.0, ml_dtypes.bfloat16)
    c = 0
    for ph, sizes in ((0, sizes_a), (1, sizes_b)):
        off = 0
        for size in sizes:
            tok = slice(off, off + size)
            gidx[c, :, :size // 16] = _wrap16(g_all[ph, tok], size)
            # token t of chunk -> [t % 128, t // 128]
            wl[c, :, :size // 128] = (
                w_all[ph, tok].reshape(size // 128, 128).T
                .astype(ml_dtypes.bfloat16))
            scol[c, :, :size // 128] = (
                s_all[ph, tok].reshape(size // 128, 128).T
                .astype(ml_dtypes.bfloat16))
            off += size
            c += 1
    return {"gidx": gidx, "wl": wl, "scol": scol}


def kernel(**inputs):
    H = np.ascontiguousarray(np.asarray(inputs["H"], np.float32))
    w = np.asarray(inputs["edge_w"], np.float32)
    src = np.asarray(inputs["edge_src"], np.int64)
    dst = np.asarray(inputs["edge_dst"], np.int64)

    edges = []
    worst = 1
    for core in range(8):
        b, s = core // 2, core % 2
        srcs, dloc, ws, phase = _core_edges(src[b], dst[b], w[b], s)
        edges.append((srcs, dloc, ws, phase))
        worst = max(worst, int((phase == 0).sum()), int((phase == 1).sum()))

    # pack all cores; grow nbins until feasible everywhere
    nbins = max(-(-NHALF // W), -(-int(worst * 1.01) // CAP))
    nbins = -(-nbins // 8) * 8
    while True:
        metas = []
        for core in range(8):
            srcs, dloc, ws, phase = edges[core]
            dA = np.bincount(srcs[phase == 0], minlength=NHALF)
            dB = np.bincount(srcs[phase == 1], minlength=NHALF)
            res = _pack_bins(dA, dB, nbins)
            if res is None:
                break
            metas.append(res)
        if len(metas) == 8:
            break
        nbins += 8
    nch_per_phase = -(-(nbins * CAP) // CH)

    iotab = np.tile(np.arange(W), (128, 1)).astype(ml_dtypes.bfloat16)

    in_maps = []
    for core in range(8):
        b = core // 2
        srcs, dloc, ws, phase = edges[core]
        bin_of, slot_of = metas[core]
        m = _prep_core(srcs, dloc, ws, phase, bin_of, slot_of, nbins,
                       nch_per_phase)
        hb = np.zeros((N, 128), ml_dtypes.bfloat16)
        hb[:, 0:HS] = H[b].astype(ml_dtypes.bfloat16)
        m["h"] = hb
        m["iotab"] = iotab
        in_maps.append(m)

    nc = _get_compiled(nbins, nch_per_phase)
    trace = bool(int(os.environ.get("GNN_TRACE", "0")))
    res = run_bass_kernel_spmd(nc, in_maps, list(range(8)), trace=trace)
    LAST_RESULT["exec_time_ns"] = res.exec_time_ns
    LAST_RESULT["res"] = res

    out = np.empty((B, N, HS), np.float32)
    rows = np.arange(NHALF)
    for core in range(8):
        b, s = core // 2, core % 2
        bin_of, slot_of = metas[core]
        dump = np.asarray(res.results[core]["acc"], np.float32)
        # acc [128, 2, nquad, 64]: row src -> partition 32*(bin%4)+slot,
        # quad bin//4; phases add.
        part = 32 * (bin_of[rows] % 4) + slot_of[rows]
        quad = bin_of[rows] // 4
        out[b, s * NHALF:(s + 1) * NHALF] = (
            dump[part, 0, quad] + dump[part, 1, quad])
    return out


# revision 5
# speedup vs baseline: 1.0491x; 1.0283x over previous
"""Neighbor aggregation (gnn message passing) Bass kernel for Trainium2.

out[b, i] = sum_{e: src[e]==i} w[e] * H[b, dst[e]]   (per batch b)

8 NeuronCores: core = 2*b + s handles batch b, src-half s (output rows
[s*25000, (s+1)*25000)).

Strategy: SWDGE dma_gather (per-edge descriptor generation on the Pool
engine, ~2.9 ns/token) is the hard floor; everything else is arranged to
stay far below it so the wall clock is pure Pool time:
 - H is stored bf16 padded to 128 cols (256B rows = the SWDGE minimum
   elem); gathered messages feed the Tensor engine DIRECTLY.  The old
   kernel's DVE MULTIPLY (w * msgs, 217us) and f32->bf16 conversion are
   gone - the edge weight is folded into the one-hot instead.
 - Bins are 32 sources wide (cap 256 tokens per phase, TPB=2).  The
   weighted one-hot ("whot") is built on DVE 32 slots wide in batches of
   16 tiles: is_equal(slot-id, iota32) then mult by w.  At 32 wide this
   is ~4x cheaper than the old 128-wide IS_EQ (893us -> ~250us), taking
   DVE off the critical path entirely (old kernel: DVE 95% busy,
   co-critical with Pool).
 - 4 bins share one PSUM tile at partition offsets 32*(bin%4) via
   explicit matmul tile_position; a DVE copy drains each 4-bin group to
   a staging tile, DMA'd to HBM every 8 bins.  The two dst-half phases
   write separate dumps; the host adds them during the un-permute.
 - Same chunk ramp / 4-SWDGE-queue rotation as before (keeps descriptor
   generation at its measured best rate).
"""

import os
import sys

sys.path.insert(0, "/opt/trn_rl_repo")

import numpy as np
import ml_dtypes

import concourse.bacc as bacc
import concourse.mybir as mybir
import concourse.tile as tile
from concourse.bass_utils import run_bass_kernel_spmd

B, N, E, HS = 4, 50000, 800000, 64
NHALF = N // 2                  # 25000 output rows per core
CH = 12288                      # tokens per gather chunk (Q7 scratch caps
                                # dma_gather at <16368 idxs: 4B/idx of 64KB)
TPB = 2                         # tiles (of 128 tokens) per bin per phase
CAP = TPB * 128                 # 256 tokens per bin per phase
W = 32                          # sources (slots) per bin

LAST_RESULT = {}


def _chunk_sizes(nbins, nch_per_phase):
    """Per-phase chunk sizes: graduated ramp first resp. last so the 4 SWDGE
    queue contexts fill/drain staggered instead of all big generations
    starting at once.  One mid-stream chunk is trimmed so the region covers
    exactly nbins*CAP tokens (no dummy-tile pad)."""
    trim = nch_per_phase * CH - nbins * CAP         # multiple of 128, < CH
    ramp = [CH // 4] * 4 + [CH // 2] * 2            # = 2*CH
    sizes_a = ramp + [CH] * (nch_per_phase - 3) + [CH - trim]
    sizes_b = [CH - trim] + [CH] * (nch_per_phase - 3) + ramp[::-1]
    sizes_a = [s for s in sizes_a if s > 0]
    sizes_b = [s for s in sizes_b if s > 0]
    return sizes_a, sizes_b


def build(nc, nbins, nch_per_phase):
    f32 = mybir.dt.float32
    bf16 = mybir.dt.bfloat16
    i16 = mybir.dt.int16
    sizes_a, sizes_b = _chunk_sizes(nbins, nch_per_phase)
    sizes = sizes_a + sizes_b
    nch = len(sizes)
    nquad = nbins // 4

    h_d = nc.dram_tensor("h", [N, 128], bf16, kind="ExternalInput")
    gidx_d = nc.dram_tensor("gidx", [nch, 128, CH // 16], i16,
                            kind="ExternalInput")
    wl_d = nc.dram_tensor("wl", [nch, 128, CH // 128], bf16,
                          kind="ExternalInput")
    scol_d = nc.dram_tensor("scol", [nch, 128, CH // 128], bf16,
                            kind="ExternalInput")
    iotab_d = nc.dram_tensor("iotab", [128, W], bf16, kind="ExternalInput")
    acc_d = nc.dram_tensor("acc", [128, 2, nquad, HS], f32,
                           kind="ExternalOutput")

    with tile.TileContext(nc) as tc:
        with tc.tile_pool(name="res", bufs=1) as res, \
             tc.tile_pool(name="psum", bufs=8, space="PSUM") as pp, \
             tc.tile_pool(name="wk", bufs=5) as wk, \
             tc.tile_pool(name="oh", bufs=4) as ohp, \
             tc.tile_pool(name="st", bufs=2) as stp:
            iotab = res.tile([128, W], bf16, tag="iotab")
            nc.sync.dma_start(iotab[:], iotab_d[:])

            ps = None
            stage = None
            off = 0                     # token offset within the phase
            for c, size in enumerate(sizes):
                phase = 0 if c < len(sizes_a) else 1
                if c == len(sizes_a):
                    off = 0
                h_ap = h_d[:][phase * NHALF:(phase + 1) * NHALF, :]
                ntile = size // 128
                gi = wk.tile([128, size // 16], i16, tag="gi")
                nc.scalar.dma_start(gi[:], gidx_d[c][:, :size // 16])
                wl = wk.tile([128, ntile], bf16, tag="wl")
                nc.scalar.dma_start(wl[:], wl_d[c][:, :ntile])
                sc = wk.tile([128, ntile], bf16, tag="sc")
                nc.scalar.dma_start(sc[:], scol_d[c][:, :ntile])

                msgs = wk.tile([128, ntile, 128], bf16, tag="msgs")
                nc.gpsimd.dma_gather(
                    out_ap=msgs[:],
                    in_ap=h_ap,
                    idxs_ap=gi[:],
                    num_idxs=size,
                    num_idxs_reg=size,
                    elem_size=128,
                    single_packet=False,
                    queue_num=c % 4,
                )

                tau0 = off // 128
                for j0 in range(0, ntile, 32):
                    nb = min(32, ntile - j0)
                    oh = ohp.tile([128, 32, W], bf16, tag="oh")
                    nc.vector.tensor_tensor(
                        out=oh[:][:, :nb, :],
                        in0=sc[:][:, j0:j0 + nb].unsqueeze(2)
                            .broadcast_to([128, nb, W]),
                        in1=iotab[:].unsqueeze(1).broadcast_to([128, nb, W]),
                        op=mybir.AluOpType.is_equal,
                    )
                    nc.vector.tensor_tensor(
                        out=oh[:][:, :nb, :],
                        in0=oh[:][:, :nb, :],
                        in1=wl[:][:, j0:j0 + nb].unsqueeze(2)
                            .broadcast_to([128, nb, W]),
                        op=mybir.AluOpType.mult,
                    )
                    for j in range(j0, j0 + nb):
                        tau = tau0 + j              # tile idx in phase
                        bin_, pos = tau // TPB, tau % TPB
                        k = bin_ % 4
                        if pos == 0 and k == 0:
                            ps = pp.tile([128, HS], f32, tag="ps")
                        nc.tensor.matmul(
                            ps[:][32 * k:32 * k + 32, :],
                            oh[:][:, j - j0, :],
                            msgs[:][:, j, 0:HS],
                            start=(pos == 0),
                            stop=(pos == TPB - 1),
                            tile_position=(0, 32 * k),
                        )
                        if pos == TPB - 1 and k == 3:
                            quad = bin_ // 4
                            if quad % 2 == 0:
                                stage = stp.tile([128, 2, HS], f32,
                                                 tag="stage")
                            nc.vector.tensor_scalar_add(
                                stage[:][:, quad % 2, :], ps[:], 0.0)
                            if quad % 2 == 1:
                                nc.sync.dma_start(
                                    acc_d[:][:, phase, quad - 1:quad + 1, :],
                                    stage[:],
                                )
                off += size
    return nc


_COMPILED = {}


def _get_compiled(nbins, nch_per_phase):
    key = (nbins, nch_per_phase)
    if key not in _COMPILED:
        nc = bacc.Bacc(
            "TRN2", target_bir_lowering=False, debug=False, num_swdge_queues=4
        )
        build(nc, nbins, nch_per_phase)
        nc.compile()
        _COMPILED[key] = nc
    return _COMPILED[key]


def _pack_bins(dA, dB, nbins):
    """Assign each source to a bin s.t. per-bin source count <= W and
    per-bin token sums <= CAP in BOTH phases."""
    nsrc = dA.shape[0]
    order = np.argsort(-(dA + dB), kind="stable")
    loadA = np.zeros(nbins, np.int64)
    loadB = np.zeros(nbins, np.int64)
    cnt = np.zeros(nbins, np.int64)
    bin_of = np.empty(nsrc, np.int64)
    slot_of = np.empty(nsrc, np.int64)
    for s in order:
        headA = CAP - loadA - dA[s]
        headB = CAP - loadB - dB[s]
        score = np.minimum(headA, headB)
        score[cnt >= W] = -1
        b = int(np.argmax(score))
        if score[b] < 0:
            return None
        bin_of[s] = b
        slot_of[s] = cnt[b]
        loadA[b] += dA[s]
        loadB[b] += dB[s]
        cnt[b] += 1
    return bin_of, slot_of


def _wrap16(idx, n):
    a = idx.reshape(n // 16, 16).T.astype(np.int16)   # [16, n//16]
    return np.ascontiguousarray(np.tile(a, (8, 1)))   # [128, n//16]


def _core_edges(src, dst, w, s):
    sel = (src >= NHALF) == bool(s)
    srcs = (src[sel] - s * NHALF).astype(np.int64)
    dsts = dst[sel].astype(np.int64)
    ws = w[sel].astype(np.float32)
    phase = (dsts >= NHALF).astype(np.int64)
    dloc = dsts - phase * NHALF
    return srcs, dloc, ws, phase


def _prep_core(srcs, dloc, ws, phase, bin_of, slot_of, nbins, nch_per_phase):
    """Build gidx/wl/scol chunk arrays for one core (batch half s)."""
    ntok = nch_per_phase * CH
    g_all = np.zeros((2, ntok), np.int64)
    w_all = np.zeros((2, ntok), np.float32)
    s_all = np.full((2, ntok), -1.0, np.float32)

    for ph in range(2):
        m = phase == ph
        sp, dp, wp_ = srcs[m], dloc[m], ws[m]
        b = bin_of[sp]
        order = np.argsort(b, kind="stable")
        sp, dp, wp_, b = sp[order], dp[order], wp_[order], b[order]
        cnts = np.bincount(b, minlength=nbins)
        starts = np.concatenate([[0], np.cumsum(cnts[:-1])])
        offs = np.arange(sp.shape[0]) - np.repeat(starts, cnts)
        pos = b * CAP + offs
        assert (offs < CAP).all()
        g_all[ph, pos] = dp
        w_all[ph, pos] = wp_
        s_all[ph, pos] = slot_of[sp]

    sizes_a, sizes_b = _chunk_sizes(nbins, nch_per_phase)
    nch = len(sizes_a) + len(sizes_b)
    gidx = np.zeros((nch, 128, CH // 16), np.int16)
    wl = np.zeros((nch, 128, CH // 128), ml_dtypes.bfloat16)
    scol = np.full((nch, 128, CH // 128), -1.0, ml_dtypes.bfloat16)
    c = 0
    for ph, sizes in ((0, sizes_a), (1, sizes_b)):
        off = 0
        for size in sizes:
            tok = slice(off, off + size)
            gidx[c, :, :size // 16] = _wrap16(g_all[ph, tok], size)
            # token t of chunk -> [t % 128, t // 128]
            wl[c, :, :size // 128] = (
                w_all[ph, tok].reshape(size // 128, 128).T
                .astype(ml_dtypes.bfloat16))
            scol[c, :, :size // 128] = (
                s_all[ph, tok].reshape(size // 128, 128).T
                .astype(ml_dtypes.bfloat16))
            off += size
            c += 1
    return {"gidx": gidx, "wl": wl, "scol": scol}


def kernel(**inputs):
    H = np.ascontiguousarray(np.asarray(inputs["H"], np.float32))
    w = np.asarray(inputs["edge_w"], np.float32)
    src = np.asarray(inputs["edge_src"], np.int64)
    dst = np.asarray(inputs["edge_dst"], np.int64)

    edges = []
    worst = 1
    for core in range(8):
        b, s = core // 2, core % 2
        srcs, dloc, ws, phase = _core_edges(src[b], dst[b], w[b], s)
        edges.append((srcs, dloc, ws, phase))
        worst = max(worst, int((phase == 0).sum()), int((phase == 1).sum()))

    # pack all cores; grow nbins until feasible everywhere
    nbins = max(-(-NHALF // W), -(-int(worst * 1.01) // CAP))
    nbins = -(-nbins // 8) * 8
    while True:
        metas = []
        for core in range(8):
            srcs, dloc, ws, phase = edges[core]
            dA = np.bincount(srcs[phase == 0], minlength=NHALF)
            dB = np.bincount(srcs[phase == 1], minlength=NHALF)
            res = _pack_bins(dA, dB, nbins)
            if res is None:
                break
            metas.append(res)
        if len(metas) == 8:
            break
        nbins += 8
    nch_per_phase = -(-(nbins * CAP) // CH)

    iotab = np.tile(np.arange(W), (128, 1)).astype(ml_dtypes.bfloat16)

    in_maps = []
    for core in range(8):
        b = core // 2
        srcs, dloc, ws, phase = edges[core]
        bin_of, slot_of = metas[core]
        m = _prep_core(srcs, dloc, ws, phase, bin_of, slot_of, nbins,
                       nch_per_phase)
        hb = np.zeros((N, 128), ml_dtypes.bfloat16)
        hb[:, 0:HS] = H[b].astype(ml_dtypes.bfloat16)
        m["h"] = hb
        m["iotab"] = iotab
        in_maps.append(m)

    nc = _get_compiled(nbins, nch_per_phase)
    trace = bool(int(os.environ.get("GNN_TRACE", "0")))
    res = run_bass_kernel_spmd(nc, in_maps, list(range(8)), trace=trace)
    LAST_RESULT["exec_time_ns"] = res.exec_time_ns
    LAST_RESULT["res"] = res

    out = np.empty((B, N, HS), np.float32)
    rows = np.arange(NHALF)
    for core in range(8):
        b, s = core // 2, core % 2
        bin_of, slot_of = metas[core]
        dump = np.asarray(res.results[core]["acc"], np.float32)
        # acc [128, 2, nquad, 64]: row src -> partition 32*(bin%4)+slot,
        # quad bin//4; phases add.
        part = 32 * (bin_of[rows] % 4) + slot_of[rows]
        quad = bin_of[rows] // 4
        out[b, s * NHALF:(s + 1) * NHALF] = (
            dump[part, 0, quad] + dump[part, 1, quad])
    return out
